# revision 8
# baseline (speedup 1.0000x reference)
# Trainium2 Bass kernel for nn_Encoder_55370718380126 (dense transformer,
# 8 pre-norm causal encoder layers, d=1024, 8 heads, ffn 4096, b=4, n=1024+cls).
#
# Sharding: data-parallel over batch (4 elems) x tensor-parallel 2 within
# core pairs {0,1},{2,3},{4,5},{6,7} (Megatron-style: heads 4+4, d_ff
# 2048+2048), pair AllReduce after attention-out and FFN-down projections.
#
# Self-contained: hardcodes all shapes; only needs numpy + concourse (+8
# neuron cores visible through jax).
import numpy as np
import ml_dtypes
from contextlib import ExitStack

import concourse.bass as bass
import concourse.tile as tile
import concourse.mybir as mybir
from concourse.bass_utils import run_bass_kernel_spmd
from concourse.vector_clock import ScopedClock

F32 = mybir.dt.float32
BF16 = mybir.dt.bfloat16
AF = mybir.ActivationFunctionType
ALU = mybir.AluOpType

P = 128
DIM = 1024
HEADS = 8
HD = 128
NSEQ = 1025          # 1024 + cls
NPAD = 1152          # 9 * 128
T = NPAD // P        # 9 token tiles
DC = DIM // P        # 8 d chunks
H_LOC = 4            # heads per core (TP=2)
QK_LOC = H_LOC * HD  # 512
DFF_LOC = 2048       # ffn hidden per core
CGF = DFF_LOC // P   # 16 ffn col groups
EPS = 1e-5
SCALE = DIM ** -0.5
NEG = -30000.0       # additive mask value (device exp flushes to exact 0 below -100)
N_LAYERS_FULL = 8
GROUPS = [[0, 1], [2, 3], [4, 5], [6, 7]]

_prog_cache = {}


# ---------------------------------------------------------------------------
# workarounds for this container's walrus build (max 1 sem wait per inst)
# ---------------------------------------------------------------------------

def _patched_drain_and_barrier(self, tick_clock, wait_clock):
    nc = self.nc
    probe = nc.sync.nop()
    wait_clock.add_sem_waits(probe.ins, ScopedClock({None: tick_clock.global_clock}))
    si = probe.ins.sync_info
    if si is not None and len(si.on_wait) > 1:
        waits = list(si.on_wait)
        probe.ins.sync_info = mybir.SyncInfo(
            on_update=list(si.on_update), on_wait=waits[:1]
        )
        for w in waits[1:]:
            extra = nc.sync.nop()
            extra.ins.sync_info = mybir.SyncInfo(on_update=[], on_wait=[w])
    nc.sync.drain()
    nc.all_engine_barrier()
    popped = nc._tile_sem_poison_stack.pop()
    assert popped is self._sem_poison
    nc.clear_and_free_semaphores(list(self.sems.allocated().values()))
    nc.all_engine_barrier()


def _apply_patches():
    tile.TileContext._drain_and_barrier = _patched_drain_and_barrier
    try:
        from concourse import tile_utils
        if getattr(tile_utils, "max_sbuf_usage", None) is not None:
            tile_utils.max_sbuf_usage = 208 * 1024
    except Exception:
        pass


def _split_multiwait_insts(nc):
    """walrus here rejects >1 sem wait per instruction; hoist extras onto
    single-wait NOPs inserted just before, on the same engine queue."""
    ctr = 0
    for f in nc.m.functions:
        for b in f.blocks:
            new_insts = []
            changed = False
            for ins in b.instructions:
                si = ins.sync_info
                if si is not None and len(si.on_wait) > 1:
                    waits = list(si.on_wait)
                    for w in waits[:-1]:
                        nop = mybir.InstNoOp(name=f"waitfix_{ctr}", engine=ins.engine)
                        ctr += 1
                        nop.sync_info = mybir.SyncInfo(on_update=[], on_wait=[w])
                        new_insts.append(nop)
                    ins.sync_info = mybir.SyncInfo(
                        on_update=list(si.on_update), on_wait=[waits[-1]]
                    )
                    changed = True
                new_insts.append(ins)
            if changed:
                b.instructions = new_insts
    return ctr


# ---------------------------------------------------------------------------
# device program
# ---------------------------------------------------------------------------

def build_program(n_layers=N_LAYERS_FULL):
    _apply_patches()
    nc = bass.Bass(num_devices=8)

    x_in = nc.dram_tensor("x_in", [NPAD, DIM], F32, kind="ExternalInput")
    wq = nc.dram_tensor("wq", [n_layers, DIM, QK_LOC], BF16, kind="ExternalInput")
    wk = nc.dram_tensor("wk", [n_layers, DIM, QK_LOC], BF16, kind="ExternalInput")
    wv = nc.dram_tensor("wv", [n_layers, DIM, QK_LOC], BF16, kind="ExternalInput")
    bq = nc.dram_tensor("bq", [n_layers, QK_LOC], F32, kind="ExternalInput")
    bk = nc.dram_tensor("bk", [n_layers, QK_LOC], F32, kind="ExternalInput")
    bv = nc.dram_tensor("bv", [n_layers, QK_LOC], BF16, kind="ExternalInput")
    wo = nc.dram_tensor("wo", [n_layers, QK_LOC, DIM], BF16, kind="ExternalInput")
    bo = nc.dram_tensor("bo", [n_layers, DIM], BF16, kind="ExternalInput")
    w1 = nc.dram_tensor("w1", [n_layers, DIM, DFF_LOC], BF16, kind="ExternalInput")
    b1 = nc.dram_tensor("b1", [n_layers, DFF_LOC], F32, kind="ExternalInput")
    w2 = nc.dram_tensor("w2", [n_layers, DFF_LOC, DIM], BF16, kind="ExternalInput")
    b2 = nc.dram_tensor("b2", [n_layers, DIM], BF16, kind="ExternalInput")
    km_in = nc.dram_tensor("km", [P, T], F32, kind="ExternalInput")
    tri_in = nc.dram_tensor("tri", [P, P], F32, kind="ExternalInput")
    ident_in = nc.dram_tensor("ident", [P, P], BF16, kind="ExternalInput")

    x_out = nc.dram_tensor("x_out", [NPAD, DIM], F32, kind="ExternalOutput")
    cls_out = nc.dram_tensor("cls_out", [1, DIM], F32, kind="ExternalOutput")

    with tile.TileContext(nc) as tc, ExitStack() as ctx:
        # ---- persistent SBUF pools --------------------------------------
        xpool = ctx.enter_context(tc.tile_pool(name="xres", bufs=1))
        ypool = ctx.enter_context(tc.tile_pool(name="yT", bufs=1))
        bigpool = ctx.enter_context(tc.tile_pool(name="big", bufs=4))
        epool = ctx.enter_context(tc.tile_pool(name="expt", bufs=3))
        cpool = ctx.enter_context(tc.tile_pool(name="consts", bufs=1))
        spool = ctx.enter_context(tc.tile_pool(name="smalls", bufs=1))
        yscpool = ctx.enter_context(tc.tile_pool(name="ysc", bufs=1))
        wqkvpool = ctx.enter_context(tc.tile_pool(name="wqkv", bufs=2))
        wopool = ctx.enter_context(tc.tile_pool(name="wo", bufs=1))
        w1pool = ctx.enter_context(tc.tile_pool(name="w1", bufs=8))
        w2pool = ctx.enter_context(tc.tile_pool(name="w2", bufs=8))
        drampool = ctx.enter_context(tc.tile_pool(name="dram", bufs=4, space="DRAM"))

        x_sb = xpool.tile([P, T, DIM], F32)
        yT = ypool.tile([P, DC, NPAD], BF16)

        ident = cpool.tile([P, P], BF16, tag="ident")
        tri = cpool.tile([P, P], F32, tag="tri")
        km = cpool.tile([P, T], F32, tag="km")
        ones_col = cpool.tile([P, 1], BF16, tag="ones_col")
        ones_row = cpool.tile([1, P], BF16, tag="ones_row")
        ones_row_f = cpool.tile([1, P], F32, tag="ones_row_f")
        eps_t = cpool.tile([P, 1], F32, tag="eps")
        nc.sync.dma_start(ident[:], ident_in[:])
        nc.sync.dma_start(tri[:], tri_in[:])
        nc.sync.dma_start(km[:], km_in[:])
        nc.vector.memset(ones_col[:], 1.0)
        nc.vector.memset(ones_row[:], 1.0)
        nc.vector.memset(ones_row_f[:], 1.0)
        nc.vector.memset(eps_t[:], EPS)

        for t in range(T):
            nc.sync.dma_start(x_sb[:, t, :], x_in[bass.ts(t, P), :])

        def layernorm_to_yT(tag):
            """x_sb -> yT (normalized, transposed, bf16). g/b folded host-side."""
            stats = spool.tile([P, T, 2], F32, tag="stats")
            bnt = spool.tile([P, 2, 6], F32, tag="bnt")
            inv = spool.tile([P, T], F32, tag="inv")
            negminv = spool.tile([P, T], F32, tag="negminv")
            std = spool.tile([P, T], F32, tag="std")
            for t in range(T):
                for i in range(2):
                    nc.vector.bn_stats(
                        bnt[:, i, :], x_sb[:, t, bass.ts(i, 512)]
                    )
                nc.vector.bn_aggr(stats[:, t, :], bnt[:])
            nc.scalar.activation(std[:], stats[:, :, 1], AF.Sqrt, bias=eps_t[:], scale=1.0)
            nc.vector.reciprocal(inv[:], std[:])
            nc.vector.scalar_tensor_tensor(
                negminv[:], stats[:, :, 0], -1.0, inv[:], ALU.mult, ALU.mult
            )
            with tc.tile_pool(name=f"pstr_{tag}", bufs=2, space="PSUM") as trpool:
                for t in range(T):
                    ysc = yscpool.tile([P, DIM], BF16, tag="ysc")
                    nc.scalar.activation(
                        ysc[:], x_sb[:, t, :], AF.Identity,
                        bias=negminv[:, t:t + 1], scale=inv[:, t:t + 1],
                    )
                    for g in range(2):
                        ptr = trpool.tile([P, 4 * P], BF16, tag="tr")
                        for j in range(4):
                            k = g * 4 + j
                            nc.tensor.transpose(
                                ptr[:, bass.ts(j, P)], ysc[:, bass.ts(k, P)],
                                ident[:],
                            )
                        nc.scalar.copy(
                            yT[:, g * 4:(g + 1) * 4, bass.ts(t, P)],
                            ptr[:].rearrange("p (a c) -> p a c", a=4),
                        )

        def qkv_phase(li):
            """yT -> qT,kT (col-major, bias at evict) and v (token-major)."""
            wq_sb = wqkvpool.tile([P, DC, QK_LOC], BF16, tag="wqkv")
            wk_sb = wqkvpool.tile([P, DC, QK_LOC], BF16, tag="wqkv")
            nc.sync.dma_start(wq_sb[:], wq[li].rearrange("(k p) c -> p k c", p=P))
            nc.sync.dma_start(wk_sb[:], wk[li].rearrange("(k p) c -> p k c", p=P))
            bq_sb = spool.tile([P, H_LOC], F32, tag="bq")
            bk_sb = spool.tile([P, H_LOC], F32, tag="bk")
            bv_sb = spool.tile([1, QK_LOC], BF16, tag="bv")
            nc.sync.dma_start(bq_sb[:], bq[li].rearrange("(a p) -> p a", p=P))
            nc.sync.dma_start(bk_sb[:], bk[li].rearrange("(a p) -> p a", p=P))
            nc.sync.dma_start(bv_sb[:], bv[li, None, :])

            qT = bigpool.tile([P, H_LOC, NPAD], BF16, tag="big")
            kT = bigpool.tile([P, H_LOC, NPAD], BF16, tag="big")
            v_sb = bigpool.tile([P, T, QK_LOC], BF16, tag="big")

            with tc.tile_pool(name=f"psqkv_{li}", bufs=2, space="PSUM") as pspool:
                for dst, wsb, bias in ((qT, wq_sb, bq_sb), (kT, wk_sb, bk_sb)):
                    for cg in range(H_LOC):
                        for w0 in range(0, NPAD, 512):
                            wlen = min(512, NPAD - w0)
                            ps = pspool.tile([P, 512], F32, tag="ps")
                            for k in range(DC):
                                nc.tensor.matmul(
                                    ps[:, :wlen],
                                    wsb[:, k, bass.ts(cg, P)],
                                    yT[:, k, w0:w0 + wlen],
                                    start=(k == 0), stop=(k == DC - 1),
                                )
                            nc.scalar.activation(
                                dst[:, cg, w0:w0 + wlen], ps[:, :wlen], AF.Identity,
                                bias=bias[:, cg:cg + 1], scale=1.0,
                            )
                wv_sb = wqkvpool.tile([P, DC, QK_LOC], BF16, tag="wqkv")
                nc.sync.dma_start(
                    wv_sb[:], wv[li].rearrange("(k p) c -> p k c", p=P))
                for t in range(T):
                    ps = pspool.tile([P, 512], F32, tag="ps")
                    nc.tensor.matmul(ps[:], ones_row[:], bv_sb[:],
                                     start=True, stop=False,
                                     skip_group_check=True)
                    for k in range(DC):
                        nc.tensor.matmul(
                            ps[:], yT[:, k, bass.ts(t, P)], wv_sb[:, k, :],
                            start=False, stop=(k == DC - 1),
                            skip_group_check=True,
                        )
                    nc.scalar.copy(v_sb[:, t, :], ps[:])
            return qT, kT, v_sb

        def attention(qT, kT, v_sb, tag):
            oT = bigpool.tile([P, H_LOC, NPAD], BF16, tag="big")
            recip = spool.tile([P, NPAD], F32, tag="recip")
            for h in range(H_LOC):
                sums_sb = spool.tile([1, NPAD], F32, tag="sums_sb")
                with (
                    tc.tile_pool(name=f"pso_{tag}_{h}", bufs=1, space="PSUM") as pso,
                    tc.tile_pool(name=f"pss_{tag}_{h}", bufs=2, space="PSUM") as pss,
                    tc.tile_pool(name=f"psm_{tag}_{h}", bufs=1, space="PSUM") as psm,
                ):
                    psum_o = pso.tile([P, NPAD], F32, tag="pso")
                    psum_sums = psm.tile([1, NPAD], F32, tag="sums")
                    for kt in range(T):
                        expt = epool.tile([P, NPAD], BF16, tag="expt")
                        for w0 in range(kt * P, NPAD, 512):
                            wlen = min(512, NPAD - w0)
                            ps = pss.tile([P, 512], F32, tag="sc")
                            nc.tensor.matmul(
                                ps[:, :wlen],
                                kT[:, h, bass.ts(kt, P)],
                                qT[:, h, w0:w0 + wlen],
                                start=True, stop=True,
                            )
                            if w0 == kt * P:
                                nc.vector.tensor_add(ps[:, :P], ps[:, :P], tri[:])
                            nc.scalar.activation(
                                expt[:, w0:w0 + wlen], ps[:, :wlen], AF.Exp,
                                bias=km[:, kt:kt + 1], scale=1.0,
                            )
                        for w0 in range(kt * P, NPAD, 512):
                            wlen = min(512, NPAD - w0)
                            nc.tensor.matmul(
                                psum_o[:, w0:w0 + wlen],
                                v_sb[:, kt, bass.ts(h, HD)],
                                expt[:, w0:w0 + wlen],
                                start=(kt == 0), stop=(kt == T - 1),
                                skip_group_check=True,
                            )
                            nc.tensor.matmul(
                                psum_sums[:, w0:w0 + wlen],
                                ones_col[:],
                                expt[:, w0:w0 + wlen],
                                start=(kt == 0), stop=(kt == T - 1),
                                skip_group_check=True,
                            )
                    nc.scalar.copy(sums_sb[:], psum_sums[:])
                    for w0 in range(0, NPAD, 512):
                        wlen = min(512, NPAD - w0)
                        psb = pss.tile([P, 512], F32, tag="sc")
                        nc.tensor.matmul(
                            psb[:, :wlen], ones_row_f[:], sums_sb[:, w0:w0 + wlen],
                            start=True, stop=True,
                        )
                        nc.vector.reciprocal(recip[:, w0:w0 + wlen], psb[:, :wlen])
                    nc.vector.tensor_mul(oT[:, h, :], psum_o[:], recip[:])
            return oT

        def wo_phase(li, oT, cc_in):
            wo_sb = wopool.tile([P, H_LOC, DIM], BF16, tag="wo")
            nc.sync.dma_start(wo_sb[:], wo[li].rearrange("(k p) c -> p k c", p=P))
            bo_sb = spool.tile([1, DIM], BF16, tag="bo")
            nc.sync.dma_start(bo_sb[:], bo[li, None, :])
            with tc.tile_pool(name=f"psw_{li}", bufs=2, space="PSUM") as psw:
                for t in range(T):
                    ps = psw.tile([P, DIM], F32, tag="ps")
                    for cw in range(2):
                        nc.tensor.matmul(
                            ps[:, bass.ts(cw, 512)], ones_row[:],
                            bo_sb[:, bass.ts(cw, 512)],
                            start=True, stop=False, skip_group_check=True,
                        )
                        for h in range(H_LOC):
                            nc.tensor.matmul(
                                ps[:, bass.ts(cw, 512)],
                                oT[:, h, bass.ts(t, P)],
                                wo_sb[:, h, bass.ts(cw, 512)],
                                start=False, stop=(h == H_LOC - 1),
                                skip_group_check=True,
                            )
                    stg = epool.tile([P, NPAD], BF16, tag="expt")
                    nc.scalar.copy(stg[:, :DIM], ps[:])
                    nc.sync.dma_start(cc_in[bass.ts(t, P), :], stg[:, :DIM])

        def allreduce_residual(cc_in, cc_out):
            nc.gpsimd.collective_compute(
                "AllReduce", ALU.add, replica_groups=GROUPS,
                ins=[cc_in[:]], outs=[cc_out[:]],
            )
            for t in range(T):
                stg = epool.tile([P, NPAD], BF16, tag="expt")
                nc.sync.dma_start(stg[:, :DIM], cc_out[bass.ts(t, P), :])
                nc.vector.tensor_add(x_sb[:, t, :], x_sb[:, t, :], stg[:, :DIM])

        def ffn_phase(li, cc_in):
            w1_sb = [w1pool.tile([P, DFF_LOC], BF16, tag="w1", name=f"w1sb{li}_{i}") for i in range(DC)]
            for k in range(DC):
                nc.sync.dma_start(w1_sb[k][:], w1[li, bass.ts(k, P), :])
            b1_sb = spool.tile([P, CGF], F32, tag="b1")
            nc.sync.dma_start(b1_sb[:], b1[li].rearrange("(a p) -> p a", p=P))
            hT = [bigpool.tile([P, H_LOC, NPAD], BF16, tag="big", name=f"hT{li}_{i}") for i in range(4)]
            with tc.tile_pool(name=f"psf1_{li}", bufs=2, space="PSUM") as psf1:
                for cg in range(CGF):
                    ps = psf1.tile([P, NPAD], F32, tag="ps")
                    for w0 in range(0, NPAD, 512):
                        wlen = min(512, NPAD - w0)
                        for k in range(DC):
                            nc.tensor.matmul(
                                ps[:, w0:w0 + wlen],
                                w1_sb[k][:, bass.ts(cg, P)],
                                yT[:, k, w0:w0 + wlen],
                                start=(k == 0), stop=(k == DC - 1),
                            )
                    nc.scalar.activation(
                        hT[cg // 4][:, cg % 4, :], ps[:], AF.Gelu,
                        bias=b1_sb[:, cg:cg + 1], scale=1.0,
                    )
            w2_sb = [w2pool.tile([P, 2, DIM], BF16, tag="w2", name=f"w2sb{li}_{i}") for i in range(DC)]
            for k in range(DC):
                nc.sync.dma_start(
                    w2_sb[k][:],
                    w2[li, k * 2 * P:(k + 1) * 2 * P, :].rearrange(
                        "(a p) c -> p a c", p=P),
                )
            b2_sb = spool.tile([1, DIM], BF16, tag="b2")
            nc.sync.dma_start(b2_sb[:], b2[li, None, :])
            with tc.tile_pool(name=f"psf2_{li}", bufs=2, space="PSUM") as psf2:
                for t in range(T):
                    ps = psf2.tile([P, DIM], F32, tag="ps")
                    for cw in range(2):
                        nc.tensor.matmul(
                            ps[:, bass.ts(cw, 512)], ones_row[:],
                            b2_sb[:, bass.ts(cw, 512)],
                            start=True, stop=False, skip_group_check=True,
                        )
                        for kk in range(CGF):
                            nc.tensor.matmul(
                                ps[:, bass.ts(cw, 512)],
                                hT[kk // 4][:, kk % 4, bass.ts(t, P)],
                                w2_sb[kk // 2][:, kk % 2, bass.ts(cw, 512)],
                                start=False, stop=(kk == CGF - 1),
                                skip_group_check=True,
                            )
                    stg = epool.tile([P, NPAD], BF16, tag="expt")
                    nc.scalar.copy(stg[:, :DIM], ps[:])
                    nc.sync.dma_start(cc_in[bass.ts(t, P), :], stg[:, :DIM])

        for li in range(n_layers):
            layernorm_to_yT(f"ln1_{li}")
            qT, kT, v_sb = qkv_phase(li)
            oT = attention(qT, kT, v_sb, f"a{li}")
            cc_in = drampool.tile([NPAD, DIM], BF16, tag="cc")
            cc_out = drampool.tile([NPAD, DIM], BF16, tag="cc")
            wo_phase(li, oT, cc_in)
            allreduce_residual(cc_in, cc_out)
            layernorm_to_yT(f"ln2_{li}")
            cc_in = drampool.tile([NPAD, DIM], BF16, tag="cc")
            cc_out = drampool.tile([NPAD, DIM], BF16, tag="cc")
            ffn_phase(li, cc_in)
            allreduce_residual(cc_in, cc_out)
            if li == n_layers // 2 - 1:
                nc.sync.dma_start(cls_out[:], x_sb[0:1, 0, :])

        for t in range(T):
            nc.sync.dma_start(x_out[bass.ts(t, P), :], x_sb[:, t, :])

    _split_multiwait_insts(nc)
    return nc


# ---------------------------------------------------------------------------
# host side: shard, run, gather
# ---------------------------------------------------------------------------

def _bf16(a):
    return np.asarray(a, dtype=ml_dtypes.bfloat16)


def _make_core_inputs(c, n_layers, x, src_mask, cls, stacks):
    b, p = c // 2, c % 2
    ln1_g, ln1_b, wqkv, wo_, bo_, ln2_g, ln2_b, w1_, b1_, w2_, b2_ = stacks

    xb = np.concatenate([np.broadcast_to(cls[0], (1, DIM)), x[b]], axis=0)
    x_pad = np.zeros((NPAD, DIM), np.float32)
    x_pad[:NSEQ] = xb

    maskp = np.concatenate([[True], src_mask[b]])
    kmv = np.where(maskp, 0.0, NEG).astype(np.float32)
    km_full = np.full(NPAD, NEG, np.float32)
    km_full[:NSEQ] = kmv
    km_host = np.ascontiguousarray(km_full.reshape(T, P).T)   # [P, T]

    tri_host = np.where(
        np.arange(P)[:, None] > np.arange(P)[None, :], NEG, 0.0
    ).astype(np.float32)
    ident_host = _bf16(np.eye(P, dtype=np.float32))

    hsl = slice(p * H_LOC, (p + 1) * H_LOC)
    fsl = slice(p * DFF_LOC, (p + 1) * DFF_LOC)

    L = n_layers
    wqs = np.empty((L, DIM, QK_LOC), ml_dtypes.bfloat16)
    wks = np.empty((L, DIM, QK_LOC), ml_dtypes.bfloat16)
    wvs = np.empty((L, DIM, QK_LOC), ml_dtypes.bfloat16)
    bqs = np.empty((L, QK_LOC), np.float32)
    bks = np.empty((L, QK_LOC), np.float32)
    bvs = np.empty((L, QK_LOC), ml_dtypes.bfloat16)
    wos = np.empty((L, QK_LOC, DIM), ml_dtypes.bfloat16)
    bos = np.empty((L, DIM), ml_dtypes.bfloat16)
    w1s = np.empty((L, DIM, DFF_LOC), ml_dtypes.bfloat16)
    b1s = np.empty((L, DFF_LOC), np.float32)
    w2s = np.empty((L, DFF_LOC, DIM), ml_dtypes.bfloat16)
    b2s = np.empty((L, DIM), ml_dtypes.bfloat16)

    for l in range(L):
        wqkv_eff = ln1_g[l][:, None] * wqkv[l]            # [1024, 3072]
        bqkv_eff = ln1_b[l] @ wqkv[l]                     # [3072]
        w3 = wqkv_eff.reshape(DIM, 3, HEADS, HD)
        b3 = bqkv_eff.reshape(3, HEADS, HD)
        wqs[l] = _bf16(w3[:, 0, hsl].reshape(DIM, QK_LOC) * SCALE)
        wks[l] = _bf16(w3[:, 1, hsl].reshape(DIM, QK_LOC))
        wvs[l] = _bf16(w3[:, 2, hsl].reshape(DIM, QK_LOC))
        bqs[l] = (b3[0, hsl].reshape(QK_LOC) * SCALE).astype(np.float32)
        bks[l] = b3[1, hsl].reshape(QK_LOC).astype(np.float32)
        bvs[l] = _bf16(b3[2, hsl].reshape(QK_LOC))
        wos[l] = _bf16(wo_[l].reshape(HEADS, HD, DIM)[hsl].reshape(QK_LOC, DIM))
        bos[l] = _bf16(bo_[l] if p == 0 else np.zeros(DIM, np.float32))
        w1_eff = ln2_g[l][:, None] * w1_[l]
        b1_eff = ln2_b[l] @ w1_[l] + b1_[l]
        w1s[l] = _bf16(w1_eff[:, fsl])
        b1s[l] = b1_eff[fsl].astype(np.float32)
        w2s[l] = _bf16(w2_[l][fsl])
        b2s[l] = _bf16(b2_[l] if p == 0 else np.zeros(DIM, np.float32))

    return {
        "x_in": x_pad,
        "wq": wqs, "wk": wks, "wv": wvs,
        "bq": bqs, "bk": bks, "bv": bvs,
        "wo": wos, "bo": bos,
        "w1": w1s, "b1": b1s, "w2": w2s, "b2": b2s,
        "km": km_host, "tri": tri_host, "ident": ident_host,
    }


def run(n_layers, x, src_mask, cls, stacks, trace=False):
    if n_layers not in _prog_cache:
        _prog_cache[n_layers] = build_program(n_layers)
    nc = _prog_cache[n_layers]
    in_maps = [
        _make_core_inputs(c, n_layers, x, src_mask, cls, stacks)
        for c in range(8)
    ]
    res = run_bass_kernel_spmd(nc, in_maps, list(range(8)), trace=trace)
    xs = np.stack([res.results[2 * b]["x_out"][:NSEQ] for b in range(4)])
    clss = np.stack([res.results[2 * b]["cls_out"][0] for b in range(4)])
    return (xs, clss), res


def kernel(x, src_mask, cls,
           h_ln1_g, h_ln1_b, h_wqkv, h_wo, h_bo, h_ln2_g, h_ln2_b,
           h_w1, h_b1, h_w2, h_b2,
           t_ln1_g, t_ln1_b, t_wqkv, t_wo, t_bo, t_ln2_g, t_ln2_b,
           t_w1, t_b1, t_w2, t_b2):
    cat = lambda a, b: np.concatenate(
        [np.asarray(a, np.float32), np.asarray(b, np.float32)], axis=0)
    stacks = (
        cat(h_ln1_g, t_ln1_g), cat(h_ln1_b, t_ln1_b), cat(h_wqkv, t_wqkv),
        cat(h_wo, t_wo), cat(h_bo, t_bo), cat(h_ln2_g, t_ln2_g),
        cat(h_ln2_b, t_ln2_b), cat(h_w1, t_w1), cat(h_b1, t_b1),
        cat(h_w2, t_w2), cat(h_b2, t_b2),
    )
    (xs, clss), _ = run(
        N_LAYERS_FULL, np.asarray(x, np.float32),
        np.asarray(src_mask), np.asarray(cls, np.float32), stacks,
    )
    return (xs, clss)
